# revision 20
# baseline (speedup 1.0000x reference)
"""GQA attention (B=2, S=2048, D=2048, H=16, KV=4, HD=128) with RoPE + causal
softmax + output projection, on 8 TRN2 NeuronCores.

Sharding: B x KV = 2 x 4 = 8 perfectly balanced shards. Core c handles batch
c//4 and kv-group c%4 (4 q heads + 1 kv head). wq/wk/wv are column-sharded,
wo row-sharded; the 4 partial wo outputs per batch are summed on the host
(the unshard step for a row-sharded matmul).

Design (v2, from the 306us v1 baseline):
  - x is transposed on the HOST, so x^T tiles arrive via plain contiguous
    DMAs at full HBM bandwidth (the v1 hardware transpose crossbar at
    ~13 GB/s/queue caused 38us of PE stalls).
  - all matmul operands bf16 (f32 PSUM accumulation); FWL weight loads.
  - Q projection runs dt-outer over head PAIRS so the first matmuls only
    need one wq chunk + one x^T chunk (~1 MB) instead of 4 MB: the per-chunk
    SBUF tiles give precise DMA->matmul dependencies.
  - causal mask applied by a DVE add of a -1e30 upper-triangle into the
    score PSUM band BEFORE exp (v1 used a GpSimd multiply after exp, which
    sat in the exp->l/PV dependency chain).
  - stage4 (wo projection) of block sb is interleaved INTO stage3 of block
    sb+1, 4 units per head, each unit emitted after the next head's score
    prologue: the wo matmuls fill the PE while ACT runs the next head's
    exps, and the PSUM->SBUF output copies (on ACT) spread out instead of
    queueing 16-deep in front of the next block's exps.
  - activations stay transposed [feature, seq]: projections -> RoPE (DVE
    multiplies from PSUM + GpSimd add/sub) -> scores^T -> exp on ACT (bf16
    out, no max subtraction: scores are O(1) by construction) -> l via
    all-ones [128,128] matmul (broadcasts sum_k P into all partitions,
    PSUM-accumulated) and O^T = V.T @ P^T -> normalize O^T by 1/l
    (reciprocal_approx_fast straight from PSUM) -> out = O^T.T @ wo
    accumulated over heads, written as bf16 partials summed on host.
"""
import os
import sys

import numpy as np

if "/opt/trn_rl_repo" not in sys.path:
    sys.path.insert(0, "/opt/trn_rl_repo")

B, S, D = 2, 2048, 2048
H, KV, HD = 16, 4, 128
NREP = H // KV            # 4 q heads per core
EG = NREP * HD            # 512: per-core q width
NC_CORES = 8
SB = 4                    # seq blocks of 512
ST = 4                    # 128-row seq tiles per block
DT = D // 128             # 16 contraction tiles
SCALE = float(1.0 / np.sqrt(HD))

_CACHE = {}
LAST_RESULT = None        # BassKernelResults of the most recent run (for test.py)


def _install_trace_shim():
    """antenv.axon_hooks is missing in this image; run_bass_kernel_spmd's
    trace path needs it. Also neuter the S3 artifact upload."""
    import types

    try:
        import antenv.axon_hooks  # noqa: F401
    except ImportError:
        try:
            import antenv
            from trn_agent_boot.trn_boot import _ntff_profile_via_ctypes

            mod = types.ModuleType("antenv.axon_hooks")
            _hook = [None]
            mod.set_axon_ntff_profile_hook = lambda h: _hook.__setitem__(0, h)
            mod.get_axon_ntff_profile_hook = lambda: _hook[0]
            sys.modules["antenv.axon_hooks"] = mod
            antenv.axon_hooks = mod
            mod.set_axon_ntff_profile_hook(
                _ntff_profile_via_ctypes("/opt/axon/libaxon_pjrt.so")
            )
        except Exception:
            return
    import concourse.bass_utils as bu

    bu.upload_artifacts = lambda tmpdir: f"local:{tmpdir}"


def _build():
    import concourse.mybir as mybir
    import concourse.tile as tile
    from concourse import bacc

    f32 = mybir.dt.float32
    bf16 = mybir.dt.bfloat16
    EXP = mybir.ActivationFunctionType.Exp

    nc = bacc.Bacc(None, target_bir_lowering=False)
    x_d = nc.declare_dram_parameter("xT", [D, S], bf16, isOutput=False)
    wq_d = nc.declare_dram_parameter("wq", [D, EG], bf16, isOutput=False)
    wk_d = nc.declare_dram_parameter("wk", [D, HD], bf16, isOutput=False)
    wv_d = nc.declare_dram_parameter("wv", [D, HD], bf16, isOutput=False)
    wo_d = nc.declare_dram_parameter("wo", [EG, D], bf16, isOutput=False)
    cs_d = nc.declare_dram_parameter("csT", [128, S], f32, isOutput=False)
    tri_d = nc.declare_dram_parameter("trineg", [128, 128], bf16, isOutput=False)
    ones_d = nc.declare_dram_parameter("ones", [128, 128], bf16, isOutput=False)
    out_d = nc.declare_dram_parameter("out", [S, D], bf16, isOutput=True)

    with tile.TileContext(nc) as tc:
        with (
            tc.tile_pool(name="fixed", bufs=1) as fixed,
            tc.tile_pool(name="xt", bufs=3) as xtp,
            tc.tile_pool(name="qt", bufs=2) as qtp,
            tc.tile_pool(name="ot", bufs=2) as otp,
            tc.tile_pool(name="pt", bufs=4) as ptp,
            tc.tile_pool(name="rope", bufs=3) as ropep,
            tc.tile_pool(name="cs", bufs=4) as csp,
            tc.tile_pool(name="vt", bufs=2) as vtp,
            tc.tile_pool(name="r", bufs=2) as rp,
            tc.tile_pool(name="ob", bufs=3) as obp,
            # PSUM banks: psA holds tags proj(2)+o(1), psS st(4), psB l(1) = 8
            tc.tile_pool(name="psA", bufs=2, space="PSUM") as psA,
            tc.tile_pool(name="psS", bufs=4, space="PSUM") as psS,
            tc.tile_pool(name="psB", bufs=1, space="PSUM") as psB,
        ):
            # ---- persistent tiles ----
            # wq as 8 separate chunk tiles so the first Q matmuls wait only
            # on their own 512 KB chunk, not the whole 2 MB weight.
            wq_s = [
                fixed.tile([128, 2, EG], bf16, name=f"wq_s{i}") for i in range(8)
            ]
            wk_s = fixed.tile([128, DT, HD], bf16)
            wv_s = fixed.tile([128, DT, HD], bf16)
            wo_s = fixed.tile([128, NREP, D], bf16)
            tri_s = fixed.tile([128, 128], bf16)
            # all-ones [128,128]: the l-matmul broadcasts sum_k P into every
            # output partition at the same cost as an M=1 matmul (cost ~ N),
            # making 1/l directly consumable by the O^T normalize multiply.
            ones_s = fixed.tile([128, 128], bf16)
            KT = fixed.tile([128, SB, 512], bf16)   # rotated K^T [hd, s]
            V = fixed.tile([128, DT, HD], bf16)     # V [s%128, s-tile, hd]

            def rope(dst, psrc, cs):
                """dst[128,512] bf16 = rotate(psrc[128,512] PSUM f32).
                Rows 0:64 = real half, 64:128 = imag half (pre-permuted
                weights); cs rows 0:64 = cos^T, 64:128 = sin^T. Multiplies
                on DVE (PSUM reads), add/sub on GpSimd (SBUF only)."""
                re, im = psrc[0:64, :], psrc[64:128, :]
                co, si = cs[0:64, :], cs[64:128, :]
                t1 = ropep.tile([64, 512], bf16, tag="t1")
                nc.vector.tensor_mul(t1, re, co)
                t2 = ropep.tile([64, 512], bf16, tag="t2")
                nc.vector.tensor_mul(t2, im, si)
                nc.gpsimd.tensor_sub(dst[0:64, :], t1, t2)
                t3 = ropep.tile([64, 512], bf16, tag="t1")
                nc.vector.tensor_mul(t3, re, si)
                t4 = ropep.tile([64, 512], bf16, tag="t2")
                nc.vector.tensor_mul(t4, im, co)
                nc.gpsimd.tensor_add(dst[64:128, :], t3, t4)

            def load_xt(sb):
                """x^T for block sb: 4 plain DMA chunks (x pre-transposed on
                host), each its own tile for precise consumer deps."""
                xr = x_d.ap().rearrange("(t k) s -> k t s", k=128)
                chunks = []
                for dg in range(4):
                    xc = xtp.tile([128, 4, 512], bf16, tag=f"xt{dg}")
                    nc.sync.dma_start(
                        xc, xr[:, dg * 4 : (dg + 1) * 4, sb * 512 : (sb + 1) * 512]
                    )
                    chunks.append(xc)
                cs = csp.tile([128, 512], f32, tag="cs")
                nc.sync.dma_start(cs, cs_d.ap()[:, sb * 512 : (sb + 1) * 512])
                return chunks, cs

            def xa(chunks, dt):
                return chunks[dt // 4][:, dt % 4, :]

            def stage2(sb, chunks, cs):
                """Q^T/K^T/V projections + RoPE for block sb. Head-outer:
                head h+1's 16 matmuls cover head h's RoPE latency (which
                frees h's PSUM ring slot). The per-chunk wq/x^T tiles give
                the dt-loop progressive DMA waits for the prologue block."""
                qt = qtp.tile([128, NREP, 512], bf16, tag="qt")
                for h in range(NREP):
                    pq = psA.tile([128, 512], f32, tag="proj")
                    for dt in range(DT):
                        nc.tensor.matmul(
                            pq,
                            wq_s[dt // 2][:, dt % 2, h * 128 : (h + 1) * 128],
                            xa(chunks, dt),
                            start=(dt == 0),
                            stop=(dt == DT - 1),
                        )
                    rope(qt[:, h, :], pq, cs)

                pk = psA.tile([128, 512], f32, tag="proj")
                for dt in range(DT):
                    nc.tensor.matmul(
                        pk, wk_s[:, dt, :], xa(chunks, dt),
                        start=(dt == 0), stop=(dt == DT - 1),
                    )
                rope(KT[:, sb, :], pk, cs)

                pv = psA.tile([128, 512], f32, tag="proj")
                for dt in range(DT):
                    nc.tensor.matmul(
                        pv, wv_s[:, dt, :], xa(chunks, dt),
                        start=(dt == 0), stop=(dt == DT - 1),
                    )
                vt_tmp = vtp.tile([128, 512], bf16, tag="vt")
                nc.scalar.copy(vt_tmp, pv)
                nc.sync.dma_start_transpose(V[:, sb * ST : (sb + 1) * ST, :], vt_tmp)
                return qt

            def stage3(sb, qt, filler):
                """Causal attention for q-block sb, all 4 heads, with the
                previous block's wo-projection units (filler) interleaved
                4 per head AFTER the next head's score prologue."""
                ot = otp.tile([128, NREP, 512], bf16, tag="ot")
                nkt = (sb + 1) * ST
                DEPTH = 4
                fi = 0

                def kt_geo(kt):
                    r = kt - sb * ST
                    qo = 128 * r if r > 0 else 0
                    return r, qo

                def emit_st(h, kt):
                    r, qo = kt_geo(kt)
                    pst = psS.tile([128, 512], f32, tag="st")
                    nc.tensor.matmul(
                        pst[:, qo:],
                        KT[:, kt // ST, (kt % ST) * 128 : (kt % ST + 1) * 128],
                        qt[:, h, qo:],
                        start=True, stop=True,
                    )
                    if r >= 0:
                        # causal mask: add -1e30 strictly-upper triangle into
                        # the 128-col diagonal band before exp (DVE, PSUM RMW)
                        nc.vector.tensor_add(
                            pst[:, qo : qo + 128], pst[:, qo : qo + 128], tri_s
                        )
                    return pst

                sts = {}
                for kt in range(min(DEPTH, nkt)):
                    sts[kt] = emit_st(0, kt)
                for h in range(NREP):
                    po = psA.tile([128, 512], f32, tag="o", bufs=1)
                    pl = psB.tile([128, 512], f32, tag="l")
                    for kt in range(nkt):
                        r, qo = kt_geo(kt)
                        pst = sts.pop(kt)
                        pt = ptp.tile([128, 512], bf16, tag="pt")
                        nc.scalar.activation(pt[:, qo:], pst[:, qo:], EXP, scale=SCALE)
                        nc.tensor.matmul(
                            pl[:, qo:], ones_s, pt[:, qo:],
                            start=(kt == 0), stop=(kt == nkt - 1),
                        )
                        nc.tensor.matmul(
                            po[:, qo:], V[:, kt, :], pt[:, qo:],
                            start=(kt == 0), stop=(kt == nkt - 1),
                        )
                        if kt + DEPTH < nkt:
                            sts[kt + DEPTH] = emit_st(h, kt + DEPTH)
                    rb = rp.tile([128, 512], f32, tag="rb")
                    nc.vector.reciprocal_approx_fast(out=rb, in_=pl)
                    nc.vector.tensor_mul(ot[:, h, :], po, rb)
                    if h + 1 < NREP:
                        for kt in range(min(DEPTH, nkt)):
                            sts[kt] = emit_st(h + 1, kt)
                    for _ in range(4):
                        if fi < len(filler):
                            filler[fi]()
                            fi += 1
                while fi < len(filler):
                    filler[fi]()
                    fi += 1
                return ot

            def make_stage4(sb, ot):
                """wo projection for q-block sb as 16 deferred units."""
                units = []
                for db in range(4):
                    for st in range(ST):
                        def unit(db=db, st=st):
                            pw = psA.tile([128, 512], f32, tag="proj")
                            for h in range(NREP):
                                nc.tensor.matmul(
                                    pw,
                                    ot[:, h, st * 128 : (st + 1) * 128],
                                    wo_s[:, h, db * 512 : (db + 1) * 512],
                                    start=(h == 0), stop=(h == NREP - 1),
                                )
                            ob = obp.tile([128, 512], bf16, tag="ob")
                            nc.vector.tensor_copy(ob, pw)
                            row0 = (sb * ST + st) * 128
                            nc.sync.dma_start(
                                out_d.ap()[row0 : row0 + 128, db * 512 : (db + 1) * 512],
                                ob,
                            )
                        units.append(unit)
                return units

            # ---- HAM warmup: ~30 tiny matmuls on a memset tile keep the PE
            # "busy" through the DMA-gated startup window so the activity
            # monitor un-throttles the clock (1.2 -> 2.4 GHz) before the
            # first real projection matmuls land ----
            wu = fixed.tile([64, 160], bf16)
            nc.gpsimd.memset(wu, 0.0)
            pwu = psB.tile([32, 128], f32, tag="l", name="pwu")
            for _ in range(30):
                nc.tensor.matmul(pwu, wu[:, 0:32], wu[:, 32:160], start=True, stop=True)

            # ---- ordered DMA prologue, in first-consumption order: the
            # head-0 dt-loop needs wq chunk i//2 + x^T chunk i//4 as it
            # streams, so interleave them; cs/wk/wv early (RoPE + K-proj
            # consume them before the Q pass finishes), then block-1 x^T,
            # then wo ----
            wq_ap = wq_d.ap().rearrange("(t k) e -> k t e", k=128)
            xr0 = x_d.ap().rearrange("(t k) s -> k t s", k=128)
            xts0 = []
            nc.sync.dma_start(wq_s[0], wq_ap[:, 0:2, :])
            for dg in range(4):
                xc = xtp.tile([128, 4, 512], bf16, tag=f"xt{dg}", name=f"xc0_{dg}")
                nc.sync.dma_start(xc, xr0[:, dg * 4 : (dg + 1) * 4, 0:512])
                xts0.append(xc)
                for i in range(2 * dg + 1, 2 * dg + 3):
                    if i < 8:
                        nc.sync.dma_start(wq_s[i], wq_ap[:, 2 * i : 2 * i + 2, :])
                if dg == 0:
                    cs0 = csp.tile([128, 512], f32, tag="cs", name="cs0")
                    nc.sync.dma_start(cs0, cs_d.ap()[:, 0:512])
                if dg == 1:
                    nc.sync.dma_start(
                        wk_s, wk_d.ap().rearrange("(t k) e -> k t e", k=128)
                    )
                if dg == 2:
                    nc.sync.dma_start(
                        wv_s, wv_d.ap().rearrange("(t k) e -> k t e", k=128)
                    )
            xts = [(xts0, cs0)]
            nc.sync.dma_start(tri_s, tri_d.ap())
            nc.sync.dma_start(ones_s, ones_d.ap())
            xts.append(load_xt(1))
            wo_ap = wo_d.ap().rearrange("(h k) n -> k h n", k=128)
            for i in range(4):
                nc.sync.dma_start(
                    wo_s[:, :, 512 * i : 512 * i + 512],
                    wo_ap[:, :, 512 * i : 512 * i + 512],
                )
            qt = stage2(0, *xts[0])
            filler = []
            for sb in range(SB):
                if sb + 2 < SB:
                    xts.append(load_xt(sb + 2))
                ot = stage3(sb, qt, filler)
                if sb + 1 < SB:
                    qt = stage2(sb + 1, *xts[sb + 1])
                filler = make_stage4(sb, ot)
            for u in filler:
                u()
    nc.finalize()
    return nc


def _get_nc():
    if "nc" not in _CACHE:
        _CACHE["nc"] = _build()
    return _CACHE["nc"]


def _host_prep(x, wq, wk, wv, wo, freqs_cos, freqs_sin):
    """Build the 8 per-core input maps (bf16 casts + transposes on host)."""
    import ml_dtypes

    bf = ml_dtypes.bfloat16
    perm = np.concatenate([np.arange(0, HD, 2), np.arange(1, HD, 2)])  # even|odd
    csT = np.concatenate(
        [np.ascontiguousarray(freqs_cos.T), np.ascontiguousarray(freqs_sin.T)], axis=0
    ).astype(np.float32)  # [128, S]
    # trineg[kk, j] = -1e30 where j < kk (the strictly-causal-invalid part of
    # a diagonal 128-col band), 0 elsewhere
    jj = np.arange(128, dtype=np.int64)[None, :]
    kk = np.arange(128, dtype=np.int64)[:, None]
    trineg = np.where(jj < kk, np.float32(-1e30), np.float32(0.0)).astype(bf)
    ones = np.ones((128, 128), dtype=bf)

    in_maps = []
    for c in range(NC_CORES):
        b, g = divmod(c, NREP)
        wq_g = wq[:, g * EG : (g + 1) * EG].copy()
        for h in range(NREP):
            blk = wq_g[:, h * HD : (h + 1) * HD]
            wq_g[:, h * HD : (h + 1) * HD] = blk[:, perm]
        wk_g = wk[:, g * HD : (g + 1) * HD][:, perm]
        wv_g = wv[:, g * HD : (g + 1) * HD]
        wo_g = wo[g * EG : (g + 1) * EG, :]
        in_maps.append(
            {
                "xT": np.ascontiguousarray(x[b].T).astype(bf),
                "wq": np.ascontiguousarray(wq_g).astype(bf),
                "wk": np.ascontiguousarray(wk_g).astype(bf),
                "wv": np.ascontiguousarray(wv_g).astype(bf),
                "wo": np.ascontiguousarray(wo_g).astype(bf),
                "csT": csT,
                "trineg": trineg,
                "ones": ones,
            }
        )
    return in_maps


def kernel(x, wq, wk, wv, wo, freqs_cos, freqs_sin):
    global LAST_RESULT
    from concourse.bass_utils import run_bass_kernel_spmd

    trace = bool(int(os.environ.get("BASS_KERNEL_TRACE", "0")))
    if trace:
        _install_trace_shim()

    x = np.asarray(x, dtype=np.float32)
    wq = np.asarray(wq, dtype=np.float32)
    wk = np.asarray(wk, dtype=np.float32)
    wv = np.asarray(wv, dtype=np.float32)
    wo = np.asarray(wo, dtype=np.float32)
    freqs_cos = np.asarray(freqs_cos, dtype=np.float32)
    freqs_sin = np.asarray(freqs_sin, dtype=np.float32)

    nc = _get_nc()
    in_maps = _host_prep(x, wq, wk, wv, wo, freqs_cos, freqs_sin)
    res = run_bass_kernel_spmd(nc, in_maps, list(range(NC_CORES)), trace=trace)
    LAST_RESULT = res

    out = np.empty((B, S, D), dtype=np.float32)
    for b in range(B):
        acc = res.results[b * NREP]["out"].astype(np.float32)
        for g in range(1, NREP):
            acc += res.results[b * NREP + g]["out"].astype(np.float32)
        out[b] = acc
    return out


# revision 21
# speedup vs baseline: 1.0460x; 1.0460x over previous
"""GQA attention (B=2, S=2048, D=2048, H=16, KV=4, HD=128) with RoPE + causal
softmax + output projection, on 8 TRN2 NeuronCores.

Sharding: B x KV = 2 x 4 = 8 perfectly balanced shards. Core c handles batch
c//4 and kv-group c%4 (4 q heads + 1 kv head). wq/wk/wv are column-sharded,
wo row-sharded; the 4 partial wo outputs per batch are summed on the host
(the unshard step for a row-sharded matmul).

Design (v2, from the 306us v1 baseline):
  - x is transposed on the HOST, so x^T tiles arrive via plain contiguous
    DMAs at full HBM bandwidth (the v1 hardware transpose crossbar at
    ~13 GB/s/queue caused 38us of PE stalls).
  - all matmul operands bf16 (f32 PSUM accumulation); FWL weight loads.
  - Q projection runs dt-outer over head PAIRS so the first matmuls only
    need one wq chunk + one x^T chunk (~1 MB) instead of 4 MB: the per-chunk
    SBUF tiles give precise DMA->matmul dependencies.
  - causal mask applied by a DVE add of a -1e30 upper-triangle into the
    score PSUM band BEFORE exp (v1 used a GpSimd multiply after exp, which
    sat in the exp->l/PV dependency chain).
  - stage4 (wo projection) of block sb is interleaved INTO stage3 of block
    sb+1, 4 units per head, each unit emitted after the next head's score
    prologue: the wo matmuls fill the PE while ACT runs the next head's
    exps, and the PSUM->SBUF output copies (on ACT) spread out instead of
    queueing 16-deep in front of the next block's exps.
  - activations stay transposed [feature, seq]: projections -> RoPE (DVE
    multiplies from PSUM + GpSimd add/sub) -> scores^T -> exp on ACT (bf16
    out, no max subtraction: scores are O(1) by construction) -> l via
    all-ones [128,128] matmul (broadcasts sum_k P into all partitions,
    PSUM-accumulated) and O^T = V.T @ P^T -> normalize O^T by 1/l
    (reciprocal_approx_fast straight from PSUM) -> out = O^T.T @ wo
    accumulated over heads, written as bf16 partials summed on host.
"""
import os
import sys

import numpy as np

if "/opt/trn_rl_repo" not in sys.path:
    sys.path.insert(0, "/opt/trn_rl_repo")

B, S, D = 2, 2048, 2048
H, KV, HD = 16, 4, 128
NREP = H // KV            # 4 q heads per core
EG = NREP * HD            # 512: per-core q width
NC_CORES = 8
SB = 4                    # seq blocks of 512
ST = 4                    # 128-row seq tiles per block
DT = D // 128             # 16 contraction tiles
SCALE = float(1.0 / np.sqrt(HD))

_CACHE = {}
LAST_RESULT = None        # BassKernelResults of the most recent run (for test.py)


def _install_trace_shim():
    """antenv.axon_hooks is missing in this image; run_bass_kernel_spmd's
    trace path needs it. Also neuter the S3 artifact upload."""
    import types

    try:
        import antenv.axon_hooks  # noqa: F401
    except ImportError:
        try:
            import antenv
            from trn_agent_boot.trn_boot import _ntff_profile_via_ctypes

            mod = types.ModuleType("antenv.axon_hooks")
            _hook = [None]
            mod.set_axon_ntff_profile_hook = lambda h: _hook.__setitem__(0, h)
            mod.get_axon_ntff_profile_hook = lambda: _hook[0]
            sys.modules["antenv.axon_hooks"] = mod
            antenv.axon_hooks = mod
            mod.set_axon_ntff_profile_hook(
                _ntff_profile_via_ctypes("/opt/axon/libaxon_pjrt.so")
            )
        except Exception:
            return
    import concourse.bass_utils as bu

    bu.upload_artifacts = lambda tmpdir: f"local:{tmpdir}"


def _build():
    import concourse.mybir as mybir
    import concourse.tile as tile
    from concourse import bacc

    f32 = mybir.dt.float32
    bf16 = mybir.dt.bfloat16
    EXP = mybir.ActivationFunctionType.Exp

    nc = bacc.Bacc(None, target_bir_lowering=False)
    x_d = nc.declare_dram_parameter("xT", [D, S], bf16, isOutput=False)
    wq_d = nc.declare_dram_parameter("wq", [D, EG], bf16, isOutput=False)
    wk_d = nc.declare_dram_parameter("wk", [D, HD], bf16, isOutput=False)
    wv_d = nc.declare_dram_parameter("wv", [D, HD], bf16, isOutput=False)
    wo_d = nc.declare_dram_parameter("wo", [EG, D], bf16, isOutput=False)
    cs_d = nc.declare_dram_parameter("csT", [128, S], f32, isOutput=False)
    tri_d = nc.declare_dram_parameter("trineg", [128, 128], bf16, isOutput=False)
    ones_d = nc.declare_dram_parameter("ones", [128, 128], bf16, isOutput=False)
    out_d = nc.declare_dram_parameter("out", [S, D], bf16, isOutput=True)

    with tile.TileContext(nc) as tc:
        with (
            tc.tile_pool(name="fixed", bufs=1) as fixed,
            tc.tile_pool(name="xt", bufs=3) as xtp,
            tc.tile_pool(name="qt", bufs=2) as qtp,
            tc.tile_pool(name="ot", bufs=2) as otp,
            tc.tile_pool(name="pt", bufs=4) as ptp,
            tc.tile_pool(name="rope", bufs=3) as ropep,
            tc.tile_pool(name="cs", bufs=4) as csp,
            tc.tile_pool(name="vt", bufs=2) as vtp,
            tc.tile_pool(name="r", bufs=2) as rp,
            tc.tile_pool(name="ob", bufs=3) as obp,
            # PSUM banks: psA holds tags proj(2)+o(1), psS st(4), psB l(1) = 8
            tc.tile_pool(name="psA", bufs=2, space="PSUM") as psA,
            tc.tile_pool(name="psS", bufs=4, space="PSUM") as psS,
            tc.tile_pool(name="psB", bufs=1, space="PSUM") as psB,
        ):
            # ---- persistent tiles ----
            # wq as 8 separate chunk tiles so the first Q matmuls wait only
            # on their own 512 KB chunk, not the whole 2 MB weight.
            wq_s = [
                fixed.tile([128, 2, EG], bf16, name=f"wq_s{i}") for i in range(8)
            ]
            wk_s = fixed.tile([128, DT, HD], bf16)
            wv_s = fixed.tile([128, DT, HD], bf16)
            wo_s = fixed.tile([128, NREP, D], bf16)
            tri_s = fixed.tile([128, 128], bf16)
            # all-ones [128,128]: the l-matmul broadcasts sum_k P into every
            # output partition at the same cost as an M=1 matmul (cost ~ N),
            # making 1/l directly consumable by the O^T normalize multiply.
            ones_s = fixed.tile([128, 128], bf16)
            KT = fixed.tile([128, SB, 512], bf16)   # rotated K^T [hd, s]
            V = fixed.tile([128, DT, HD], bf16)     # V [s%128, s-tile, hd]

            def rope(dst, psrc, cs):
                """dst[128,512] bf16 = rotate(psrc[128,512] PSUM f32).
                Rows 0:64 = real half, 64:128 = imag half (pre-permuted
                weights); cs rows 0:64 = cos^T, 64:128 = sin^T. Multiplies
                on DVE (PSUM reads), add/sub on GpSimd (SBUF only)."""
                re, im = psrc[0:64, :], psrc[64:128, :]
                co, si = cs[0:64, :], cs[64:128, :]
                t1 = ropep.tile([64, 512], bf16, tag="t1")
                nc.vector.tensor_mul(t1, re, co)
                t2 = ropep.tile([64, 512], bf16, tag="t2")
                nc.vector.tensor_mul(t2, im, si)
                nc.gpsimd.tensor_sub(dst[0:64, :], t1, t2)
                t3 = ropep.tile([64, 512], bf16, tag="t1")
                nc.vector.tensor_mul(t3, re, si)
                t4 = ropep.tile([64, 512], bf16, tag="t2")
                nc.vector.tensor_mul(t4, im, co)
                nc.gpsimd.tensor_add(dst[64:128, :], t3, t4)

            def load_xt(sb):
                """x^T for block sb: 4 plain DMA chunks (x pre-transposed on
                host), each its own tile for precise consumer deps."""
                xr = x_d.ap().rearrange("(t k) s -> k t s", k=128)
                chunks = []
                for dg in range(4):
                    xc = xtp.tile([128, 4, 512], bf16, tag=f"xt{dg}")
                    nc.sync.dma_start(
                        xc, xr[:, dg * 4 : (dg + 1) * 4, sb * 512 : (sb + 1) * 512]
                    )
                    chunks.append(xc)
                cs = csp.tile([128, 512], f32, tag="cs")
                nc.sync.dma_start(cs, cs_d.ap()[:, sb * 512 : (sb + 1) * 512])
                return chunks, cs

            def xa(chunks, dt):
                return chunks[dt // 4][:, dt % 4, :]

            def stage2(sb, chunks, cs):
                """Q^T/K^T/V projections + RoPE for block sb. Head-outer:
                head h+1's 16 matmuls cover head h's RoPE latency (which
                frees h's PSUM ring slot). The per-chunk wq/x^T tiles give
                the dt-loop progressive DMA waits for the prologue block."""
                qt = qtp.tile([128, NREP, 512], bf16, tag="qt")
                for h in range(NREP):
                    pq = psA.tile([128, 512], f32, tag="proj")
                    for dt in range(DT):
                        nc.tensor.matmul(
                            pq,
                            wq_s[dt // 2][:, dt % 2, h * 128 : (h + 1) * 128],
                            xa(chunks, dt),
                            start=(dt == 0),
                            stop=(dt == DT - 1),
                        )
                    rope(qt[:, h, :], pq, cs)

                pk = psA.tile([128, 512], f32, tag="proj")
                for dt in range(DT):
                    nc.tensor.matmul(
                        pk, wk_s[:, dt, :], xa(chunks, dt),
                        start=(dt == 0), stop=(dt == DT - 1),
                    )
                rope(KT[:, sb, :], pk, cs)

                pv = psA.tile([128, 512], f32, tag="proj")
                for dt in range(DT):
                    nc.tensor.matmul(
                        pv, wv_s[:, dt, :], xa(chunks, dt),
                        start=(dt == 0), stop=(dt == DT - 1),
                    )
                vt_tmp = vtp.tile([128, 512], bf16, tag="vt")
                nc.scalar.copy(vt_tmp, pv)
                nc.sync.dma_start_transpose(V[:, sb * ST : (sb + 1) * ST, :], vt_tmp)
                return qt

            def stage3(sb, qt, filler):
                """Causal attention for q-block sb, all 4 heads, with the
                previous block's wo-projection units (filler) interleaved
                4 per head AFTER the next head's score prologue."""
                ot = otp.tile([128, NREP, 512], bf16, tag="ot")
                nkt = (sb + 1) * ST
                DEPTH = 4
                fi = 0

                def kt_geo(kt):
                    r = kt - sb * ST
                    qo = 128 * r if r > 0 else 0
                    return r, qo

                def emit_st(h, kt):
                    r, qo = kt_geo(kt)
                    pst = psS.tile([128, 512], f32, tag="st")
                    nc.tensor.matmul(
                        pst[:, qo:],
                        KT[:, kt // ST, (kt % ST) * 128 : (kt % ST + 1) * 128],
                        qt[:, h, qo:],
                        start=True, stop=True,
                    )
                    if r >= 0:
                        # causal mask: add -1e30 strictly-upper triangle into
                        # the 128-col diagonal band before exp (DVE, PSUM RMW)
                        nc.vector.tensor_add(
                            pst[:, qo : qo + 128], pst[:, qo : qo + 128], tri_s
                        )
                    return pst

                sts = {}
                for kt in range(min(DEPTH, nkt)):
                    sts[kt] = emit_st(0, kt)
                for h in range(NREP):
                    po = psA.tile([128, 512], f32, tag="o", bufs=1)
                    pl = psB.tile([128, 512], f32, tag="l")
                    for kt in range(nkt):
                        r, qo = kt_geo(kt)
                        pst = sts.pop(kt)
                        pt = ptp.tile([128, 512], bf16, tag="pt")
                        nc.scalar.activation(pt[:, qo:], pst[:, qo:], EXP, scale=SCALE)
                        nc.tensor.matmul(
                            pl[:, qo:], ones_s, pt[:, qo:],
                            start=(kt == 0), stop=(kt == nkt - 1),
                        )
                        nc.tensor.matmul(
                            po[:, qo:], V[:, kt, :], pt[:, qo:],
                            start=(kt == 0), stop=(kt == nkt - 1),
                        )
                        if kt + DEPTH < nkt:
                            sts[kt + DEPTH] = emit_st(h, kt + DEPTH)
                    rb = rp.tile([128, 512], f32, tag="rb")
                    nc.vector.reciprocal_approx_fast(out=rb, in_=pl)
                    nc.vector.tensor_mul(ot[:, h, :], po, rb)
                    if h + 1 < NREP:
                        for kt in range(min(DEPTH, nkt)):
                            sts[kt] = emit_st(h + 1, kt)
                    for _ in range(4):
                        if fi < len(filler):
                            filler[fi]()
                            fi += 1
                while fi < len(filler):
                    filler[fi]()
                    fi += 1
                return ot

            def make_stage4(sb, ot):
                """wo projection for q-block sb as 16 deferred units."""
                units = []
                for db in range(4):
                    for st in range(ST):
                        def unit(db=db, st=st):
                            pw = psA.tile([128, 512], f32, tag="proj")
                            for h in range(NREP):
                                nc.tensor.matmul(
                                    pw,
                                    ot[:, h, st * 128 : (st + 1) * 128],
                                    wo_s[:, h, db * 512 : (db + 1) * 512],
                                    start=(h == 0), stop=(h == NREP - 1),
                                )
                            ob = obp.tile([128, 512], bf16, tag="ob")
                            nc.vector.tensor_copy(ob, pw)
                            row0 = (sb * ST + st) * 128
                            nc.sync.dma_start(
                                out_d.ap()[row0 : row0 + 128, db * 512 : (db + 1) * 512],
                                ob,
                            )
                        units.append(unit)
                return units

            # ---- HAM warmup: ~30 tiny matmuls on a memset tile keep the PE
            # "busy" through the DMA-gated startup window so the activity
            # monitor un-throttles the clock (1.2 -> 2.4 GHz) before the
            # first real projection matmuls land ----
            wu = fixed.tile([64, 160], bf16)
            nc.gpsimd.memset(wu, 0.0)
            pwu = psB.tile([32, 128], f32, tag="l", name="pwu")
            for _ in range(30):
                nc.tensor.matmul(pwu, wu[:, 0:32], wu[:, 32:160], start=True, stop=True)

            # ---- ordered DMA prologue. DMAs round-robin over 8 hardware
            # queues in emission order (~90 GB/s each), so lay out the
            # first 8 emissions as the 4 block-0 x^T chunks + wq chunks 0-3
            # on 8 DISTINCT queues (positions 0-7), then the rest of the
            # block-0-critical set on the second rotation, then block-1 x^T
            # and wo on the third ----
            wq_ap = wq_d.ap().rearrange("(t k) e -> k t e", k=128)
            xr0 = x_d.ap().rearrange("(t k) s -> k t s", k=128)
            xts0 = []
            for dg in range(4):                              # pos 0,2,4,6
                xc = xtp.tile([128, 4, 512], bf16, tag=f"xt{dg}", name=f"xc0_{dg}")
                nc.sync.dma_start(xc, xr0[:, dg * 4 : (dg + 1) * 4, 0:512])
                xts0.append(xc)
                nc.sync.dma_start(wq_s[dg], wq_ap[:, 2 * dg : 2 * dg + 2, :])
            for i in range(4, 8):                            # pos 8-11
                nc.sync.dma_start(wq_s[i], wq_ap[:, 2 * i : 2 * i + 2, :])
            cs0 = csp.tile([128, 512], f32, tag="cs", name="cs0")
            nc.sync.dma_start(cs0, cs_d.ap()[:, 0:512])      # pos 12
            nc.sync.dma_start(wk_s, wk_d.ap().rearrange("(t k) e -> k t e", k=128))
            nc.sync.dma_start(wv_s, wv_d.ap().rearrange("(t k) e -> k t e", k=128))
            nc.sync.dma_start(tri_s, tri_d.ap())             # pos 15
            xts = [(xts0, cs0)]
            nc.sync.dma_start(ones_s, ones_d.ap())           # pos 16
            xts.append(load_xt(1))                           # pos 17-21
            wo_ap = wo_d.ap().rearrange("(h k) n -> k h n", k=128)
            for i in range(4):                               # pos 22-25
                nc.sync.dma_start(
                    wo_s[:, :, 512 * i : 512 * i + 512],
                    wo_ap[:, :, 512 * i : 512 * i + 512],
                )
            qt = stage2(0, *xts[0])
            filler = []
            for sb in range(SB):
                if sb + 2 < SB:
                    xts.append(load_xt(sb + 2))
                ot = stage3(sb, qt, filler)
                if sb + 1 < SB:
                    qt = stage2(sb + 1, *xts[sb + 1])
                filler = make_stage4(sb, ot)
            for u in filler:
                u()
    nc.finalize()
    return nc


def _get_nc():
    if "nc" not in _CACHE:
        _CACHE["nc"] = _build()
    return _CACHE["nc"]


def _host_prep(x, wq, wk, wv, wo, freqs_cos, freqs_sin):
    """Build the 8 per-core input maps (bf16 casts + transposes on host)."""
    import ml_dtypes

    bf = ml_dtypes.bfloat16
    perm = np.concatenate([np.arange(0, HD, 2), np.arange(1, HD, 2)])  # even|odd
    csT = np.concatenate(
        [np.ascontiguousarray(freqs_cos.T), np.ascontiguousarray(freqs_sin.T)], axis=0
    ).astype(np.float32)  # [128, S]
    # trineg[kk, j] = -1e30 where j < kk (the strictly-causal-invalid part of
    # a diagonal 128-col band), 0 elsewhere
    jj = np.arange(128, dtype=np.int64)[None, :]
    kk = np.arange(128, dtype=np.int64)[:, None]
    trineg = np.where(jj < kk, np.float32(-1e30), np.float32(0.0)).astype(bf)
    ones = np.ones((128, 128), dtype=bf)

    in_maps = []
    for c in range(NC_CORES):
        b, g = divmod(c, NREP)
        wq_g = wq[:, g * EG : (g + 1) * EG].copy()
        for h in range(NREP):
            blk = wq_g[:, h * HD : (h + 1) * HD]
            wq_g[:, h * HD : (h + 1) * HD] = blk[:, perm]
        wk_g = wk[:, g * HD : (g + 1) * HD][:, perm]
        wv_g = wv[:, g * HD : (g + 1) * HD]
        wo_g = wo[g * EG : (g + 1) * EG, :]
        in_maps.append(
            {
                "xT": np.ascontiguousarray(x[b].T).astype(bf),
                "wq": np.ascontiguousarray(wq_g).astype(bf),
                "wk": np.ascontiguousarray(wk_g).astype(bf),
                "wv": np.ascontiguousarray(wv_g).astype(bf),
                "wo": np.ascontiguousarray(wo_g).astype(bf),
                "csT": csT,
                "trineg": trineg,
                "ones": ones,
            }
        )
    return in_maps


def kernel(x, wq, wk, wv, wo, freqs_cos, freqs_sin):
    global LAST_RESULT
    from concourse.bass_utils import run_bass_kernel_spmd

    trace = bool(int(os.environ.get("BASS_KERNEL_TRACE", "0")))
    if trace:
        _install_trace_shim()

    x = np.asarray(x, dtype=np.float32)
    wq = np.asarray(wq, dtype=np.float32)
    wk = np.asarray(wk, dtype=np.float32)
    wv = np.asarray(wv, dtype=np.float32)
    wo = np.asarray(wo, dtype=np.float32)
    freqs_cos = np.asarray(freqs_cos, dtype=np.float32)
    freqs_sin = np.asarray(freqs_sin, dtype=np.float32)

    nc = _get_nc()
    in_maps = _host_prep(x, wq, wk, wv, wo, freqs_cos, freqs_sin)
    res = run_bass_kernel_spmd(nc, in_maps, list(range(NC_CORES)), trace=trace)
    LAST_RESULT = res

    out = np.empty((B, S, D), dtype=np.float32)
    for b in range(B):
        acc = res.results[b * NREP]["out"].astype(np.float32)
        for g in range(1, NREP):
            acc += res.results[b * NREP + g]["out"].astype(np.float32)
        out[b] = acc
    return out
